# revision 63
# baseline (speedup 1.0000x reference)
"""Trainium2 Bass kernel for nn_MC3DAD_ONNX_48146583388946 (retrieval_knn).

Per batch (one NeuronCore per batch, B=8). Distance matmuls run in bf16
with exact hi/lo coordinate splits (bf16 x bf16 products are exact in the
fp32 PSUM accumulation, and matmul cost depends only on moving columns,
not contraction depth):

  - pass A: -d^2 via a 13-row matmul (hi/lo split of the
    ||x||^2 + ||y||^2 - 2 x.y expansion); VectorE max8 -> v5 = 5th-largest
    -d^2 per point.
  - v5 is negated, split into three bf16 parts, transposed on the PE and
    scattered into three extra moving rows; pass B's 16-row matmul with
    ROLE-SWAPPED rows directly produces the TRANSPOSED margin matrix
    margin[d, q] = -d^2(d,q) - v5[q]. The row ordering makes every product
    and partial sum bit-identical to pass A's (the PE accumulates
    sequential fp32), so a tiny delta=1e-6 suffices for inclusive ties.
  - mask = Sign(margin + delta) on ScalarE, written as fp8 +-1; the
    masked sums use fp8 DoubleRow matmuls (two d-slabs per instruction).
    Features are encoded as signed BIT-PLANE rows (entries {0,+-1},
    round-to-nearest digits) so the dual-fp8 PE unit - which truncates the
    smaller pair partner on large exponent gaps - stays exact, and the
    +-1-mask sums are exact integers. Plane weights and the +-1->selection
    conversion live in a small fp32 combine matmul in the finalize.
  - finalize in [128, 32] layout: Ssq = sc-sum + cnt*mean(sq),
    trace = (Ssq - |Sxyz|^2/c)/(c-1), curvature = trace/(sum trace + 1e-8).

Coordinates are centered per batch on the host (translation-invariant
covariance); sq is centered too so every masked-sum colsum is ~0 and the
+-1 trick avoids catastrophic cancellation.

Schedule: pass A of octet k+1 and the w-row prep are software-pipelined
against pass B of octet k (PE: margins + masked sums; ScalarE: masks;
VectorE: max8), with PSUM banks split 3/2/2/1 across the four pools.
"""

import numpy as np
from contextlib import ExitStack

import concourse.bass as bass
import concourse.bacc as bacc
import concourse.mybir as mybir
import concourse.tile as tile
from concourse.bass_utils import run_bass_kernel_spmd

try:
    import ml_dtypes
    BF16 = ml_dtypes.bfloat16
    FP8 = ml_dtypes.float8_e4m3
except ImportError:  # jax ships ml_dtypes
    from jax import numpy as jnp
    BF16 = jnp.bfloat16
    FP8 = jnp.float8_e4m3

f32 = mybir.dt.float32
bf16 = mybir.dt.bfloat16
fp8 = mybir.dt.float8e4
AF = mybir.ActivationFunctionType
ALU = mybir.AluOpType
DR = mybir.MatmulPerfMode.DoubleRow

N = 4096
B = 8
DELTA = 1e-6                   # inclusive-margin nudge (abs, on d^2 scale)
KROWS = 13                     # cdist contraction rows (hi/lo split)
KROWS2 = 16                    # pass-B rows: 13 role-swapped + 3 w rows
POOL_MASKS = False             # PoolE cannot read PSUM on TRN2
PLANES_XYZ = 20                # bit planes 2^3 .. 2^-16 for x/y/z
PLANES_SC = 23                 # bit planes 2^6 .. 2^-16 for centered sq
NROWS = 3 * PLANES_XYZ + PLANES_SC + 1   # 84 digit rows per d-slab
PR = 96                        # padded to x32 (dual-fp8 LDW ISA restriction)
INTERLEAVE = True              # overlap A(oct k+1) with B(oct k)
DEBUG_S = False                # dump raw s_all to an extra output


def build_device_kernel(tc, gbt_d, gat_d, gbt2_d, gat2_d, fp18_d, comb_d,
                        id_d, c18_d, msq_d, curv_d, cnt_d, n=N, sdbg_d=None):
    nc = tc.nc
    ns = n // 128               # 128-point d-slabs (32)
    noct = ns // 8              # octets of slabs (4)
    nsc = n // 128              # finalize cols per partition (32)
    nac = n // 1024             # pass-A chunks per slab (4)
    nqc = n // 512              # 512-wide q-chunks (8)

    with ExitStack() as ctx:
        cpool = ctx.enter_context(tc.tile_pool(name="consts", bufs=1))
        gbt = cpool.tile([KROWS, n], bf16, tag="gbt")
        gat = cpool.tile([KROWS, n], bf16, tag="gat")
        gbt2 = cpool.tile([KROWS2, n], bf16, tag="gbt2")
        gat2 = cpool.tile([KROWS2, n], bf16, tag="gat2")
        fp18 = cpool.tile([128, (ns // 2) * 2 * PR], fp8, tag="fp18")
        comb = cpool.tile([PR, 5], f32, tag="comb")
        identb = cpool.tile([128, 128], bf16, tag="identb")
        c18 = cpool.tile([PR, 1], f32, tag="c18")
        msq = cpool.tile([128, 1], f32, tag="msq")
        s_all = cpool.tile([PR, n], f32, tag="s_all")
        s5 = cpool.tile([5, n], f32, tag="s5")
        dlt = cpool.tile([128, 1], f32, tag="dlt")
        ones128 = cpool.tile([128, 1], f32, tag="ones128")
        ones1 = cpool.tile([1, 128], f32, tag="ones1")
        nc.vector.memset(dlt[:, :], DELTA)
        nc.vector.memset(ones128[:, :], 1.0)
        nc.vector.memset(ones1[:, :], 1.0)

        # pass-A operands first: they gate the very first matmul; the big
        # fp18 load is only needed once pass B starts
        nc.sync.dma_start(gbt[:, :], gbt_d[:, :])
        nc.sync.dma_start(gat[:, :], gat_d[:, :])
        nc.sync.dma_start(gbt2[:, :], gbt2_d[:, :])
        nc.sync.dma_start(gat2[:, :], gat2_d[:, :])
        nc.sync.dma_start(identb[:, :], id_d[:, :])
        nc.sync.dma_start(fp18[:, :], fp18_d[:, :])
        nc.sync.dma_start(comb[:, :], comb_d[:, :])
        nc.sync.dma_start(c18[:, :], c18_d[:, :])
        nc.sync.dma_start(msq[:, :], msq_d[:, :])

        with tc.tile_pool(name="apsum", bufs=3, space="PSUM") as ap, \
             tc.tile_pool(name="mpsum", bufs=2, space="PSUM") as mp, \
             tc.tile_pool(name="spsum", bufs=2, space="PSUM") as sp, \
             tc.tile_pool(name="tpsum", bufs=1, space="PSUM") as tp, \
             tc.tile_pool(name="work", bufs=2) as wp, \
             tc.tile_pool(name="msks", bufs=4) as kp:

            v5oct = {}

            def emit_A_slab(s):
                """-d^2 row slab s -> top-8 -> v5 column in v5oct."""
                oct_, i = s // 8, s % 8
                if i == 0:
                    v5oct[oct_] = wp.tile([128, 64], f32, tag="v5o",
                                          name=f"v5o_{oct_}")
                m8 = wp.tile([128, 8 * 2 * nac], f32, tag="m8",
                             name=f"m8_{s}")
                for c in range(2 * nac):
                    d1 = ap.tile([128, 512], f32, tag="d1",
                                 name=f"d1_{s}_{c}")
                    nc.tensor.matmul(
                        d1[:, :],
                        gbt[0:KROWS, s * 128:(s + 1) * 128],
                        gat[0:KROWS, c * 512:(c + 1) * 512],
                        start=True, stop=True)
                    nc.vector.max(m8[:, c * 8:(c + 1) * 8], d1[:, :])
                nc.vector.max(v5oct[oct_][:, i * 8:(i + 1) * 8], m8[:, :])

            def emit_wprep(oct_):
                """w = -v5 for the octet's 1024 points, split into three
                bf16 parts (exact to 2^-25), transpose, scatter into gat2
                rows 13/14/15."""
                vo = v5oct[oct_]
                wf = wp.tile([128, 8], f32, tag="wf", name=f"wf_{oct_}")
                nc.vector.tensor_scalar_mul(wf[:, :], vo[:, 4:64:8], -1.0)
                wc = wp.tile([128, 24], bf16, tag="wc", name=f"wc_{oct_}")
                whf = wp.tile([128, 8], f32, tag="whf", name=f"whf_{oct_}")
                for part in range(3):
                    nc.vector.tensor_copy(wc[:, part * 8:(part + 1) * 8],
                                          wf[:, :])
                    if part == 2:
                        break
                    nc.vector.tensor_copy(whf[:, :],
                                          wc[:, part * 8:(part + 1) * 8])
                    nc.vector.tensor_sub(wf[:, :], wf[:, :], whf[:, :])
                wT = tp.tile([24, 128], bf16, tag="wT", name=f"wT_{oct_}")
                nc.tensor.transpose(wT[:, :], wc[:, :], identb[:, :])
                wTs = wp.tile([24, 128], bf16, tag="wTs", name=f"wTs_{oct_}")
                nc.vector.tensor_copy(wTs[:, :], wT[:, :])
                # [24, 128] rows (part*8+j) -> [3, 1024] row-major: same
                # linear element order, one DMA
                q0 = oct_ * 1024
                nc.sync.dma_start(gat2[13:16, q0:q0 + 1024], wTs[:, :])

            Scur = {}

            def emit_B_pair(oct_, t):
                """margin chunks for d-slab pair (2t, 2t+1) x both q-chunks
                of the octet, sharing stationaries; two DoubleRow
                masked-sum matmuls."""
                if t == 0:
                    for u in range(2):
                        qc = oct_ * 2 + u
                        Scur[qc] = sp.tile([PR, 512], f32, tag="S",
                                           name=f"S_{qc}")
                msks = {}
                for u in range(2):
                    qc = oct_ * 2 + u
                    msks[u] = kp.tile([128, 1024], fp8, tag="msk",
                                      name=f"msk_{qc}_{t}", bufs=4)
                for g in range(2):
                    d = 2 * t + g
                    for u in range(2):
                        qc = oct_ * 2 + u
                        mg = mp.tile([128, 512], f32, tag="mg",
                                     name=f"mg_{qc}_{d}")
                        nc.tensor.matmul(
                            mg[:, :],
                            gbt2[0:KROWS2, d * 128:(d + 1) * 128],
                            gat2[0:KROWS2, qc * 512:(qc + 1) * 512],
                            start=True, stop=True)
                        nc.scalar.activation(
                            msks[u][:, g * 512:(g + 1) * 512],
                            mg[:, :], AF.Sign, bias=dlt[:, 0:1])
                for u in range(2):
                    qc = oct_ * 2 + u
                    nc.tensor.matmul(
                        Scur[qc][:, :],
                        fp18[:, t * 2 * PR:(t + 1) * 2 * PR].rearrange(
                            "p (two f) -> p two f", two=2),
                        msks[u][:, :].rearrange("p (two f) -> p two f",
                                                two=2),
                        start=(t == 0), stop=(t == ns // 2 - 1),
                        perf_mode=DR)
                if t == ns // 2 - 1:
                    for u in range(2):
                        qc = oct_ * 2 + u
                        nc.scalar.copy(s_all[:, qc * 512:(qc + 1) * 512],
                                       Scur[qc][:, :])

            # ---- software-pipelined schedule ----
            # wprep(k) is emitted inside iteration k-1 right after the last
            # A-slab of octet k, so the w transpose/scatter overlaps the
            # tail of B(oct k-1) instead of stalling the octet boundary.
            for i in range(8):
                emit_A_slab(i)
            emit_wprep(0)
            for oct_ in range(noct):
                npair = ns // 2
                for i in range(8):
                    if INTERLEAVE and oct_ + 1 < noct:
                        emit_A_slab((oct_ + 1) * 8 + i)
                        if i == 7:
                            emit_wprep(oct_ + 1)
                    for t in range(i * npair // 8, (i + 1) * npair // 8):
                        emit_B_pair(oct_, t)
                if not INTERLEAVE and oct_ + 1 < noct:
                    for i in range(8):
                        emit_A_slab((oct_ + 1) * 8 + i)
                    emit_wprep(oct_ + 1)

        # ---------------- finalize ----------------
        if sdbg_d is not None:
            nc.sync.dma_start(sdbg_d[:, :], s_all[:, :])
        with tc.tile_pool(name="fin", bufs=1) as fp, \
             tc.tile_pool(name="fpsum", bufs=2, space="PSUM") as fps:
            # +-1 mask correction: + 0.5 * colsum per digit row, then fold
            # the digit rows into [x y z sq cnt] via a small fp32 matmul.
            # Per-chunk adds let chunks drained early run their correction
            # on VectorE while the PE finishes the last octet.
            for ch in range(nqc):
                sl = slice(ch * 512, (ch + 1) * 512)
                nc.vector.tensor_scalar_add(s_all[0:PR, sl],
                                            s_all[0:PR, sl], c18[0:PR, 0:1])
            # reshape rows into [128, nsc] (q = p * nsc + c, row-major);
            # per-chunk DMAs so early chunks stream out while the PE still
            # combines later ones
            fin = fp.tile([128, 5 * nsc], f32, tag="fin")
            for ch in range(nqc):
                fc = fps.tile([5, 512], f32, tag="fc", name=f"fc_{ch}")
                nc.tensor.matmul(fc[:, :], comb[:, :],
                                 s_all[:, ch * 512:(ch + 1) * 512],
                                 start=True, stop=True)
                nc.scalar.copy(s5[:, ch * 512:(ch + 1) * 512], fc[:, :])
                p0 = ch * (512 // nsc)
                p1 = p0 + 512 // nsc
                for r in range(5):
                    nc.sync.dma_start(fin[p0:p1, r * nsc:(r + 1) * nsc],
                                      s5[r:r + 1, ch * 512:(ch + 1) * 512])
            xb = fin[:, 0 * nsc:1 * nsc]
            yb = fin[:, 1 * nsc:2 * nsc]
            zb = fin[:, 2 * nsc:3 * nsc]
            sqb = fin[:, 3 * nsc:4 * nsc]
            cntb = fin[:, 4 * nsc:5 * nsc]

            # Ssq = sc-sum + cnt * msq (sq was centered host-side)
            tmp = fp.tile([128, nsc], f32, tag="tmp")
            nc.vector.tensor_scalar_mul(tmp[:, :], cntb, msq[:, 0:1])
            nc.vector.tensor_add(sqb, sqb, tmp[:, :])

            qq = fp.tile([128, nsc], f32, tag="qq")
            nc.vector.tensor_mul(qq[:, :], xb, xb)
            nc.vector.tensor_mul(tmp[:, :], yb, yb)
            nc.vector.tensor_add(qq[:, :], qq[:, :], tmp[:, :])
            nc.vector.tensor_mul(tmp[:, :], zb, zb)
            nc.vector.tensor_add(qq[:, :], qq[:, :], tmp[:, :])

            nc.sync.dma_start(cnt_d[:, :], cntb)
            rc = fp.tile([128, nsc], f32, tag="rc")
            rc1 = fp.tile([128, nsc], f32, tag="rc1")
            nc.vector.reciprocal(rc[:, :], cntb)
            cm1 = fp.tile([128, nsc], f32, tag="cm1")
            nc.vector.tensor_scalar_add(cm1[:, :], cntb, -1.0)
            nc.vector.reciprocal(rc1[:, :], cm1[:, :])

            tr = fp.tile([128, nsc], f32, tag="tr")
            nc.vector.tensor_mul(qq[:, :], qq[:, :], rc[:, :])
            nc.vector.tensor_sub(tr[:, :], sqb, qq[:, :])
            nc.vector.tensor_mul(tr[:, :], tr[:, :], rc1[:, :])

            red = fp.tile([128, 1], f32, tag="red")
            nc.vector.reduce_sum(red[:, :], tr[:, :],
                                 axis=mybir.AxisListType.X)
            # partition sum + broadcast via two tiny matmuls (avoids the
            # ~7us GpSimd library reload of partition_all_reduce)
            tot = fps.tile([1, 1], f32, tag="tot")
            nc.tensor.matmul(tot[:, :], ones128[:, :], red[:, :],
                             start=True, stop=True)
            tots = fp.tile([1, 1], f32, tag="tots")
            nc.vector.tensor_scalar_add(tots[:, :], tot[:, :], 1e-8)
            rde = fp.tile([1, 1], f32, tag="rde")
            nc.vector.reciprocal(rde[:, :], tots[:, :])
            rdb = fps.tile([128, 1], f32, tag="rdb")
            nc.tensor.matmul(rdb[:, :], ones1[:, :], rde[:, :],
                             start=True, stop=True)
            rden = fp.tile([128, 1], f32, tag="rden")
            nc.vector.tensor_copy(rden[:, :], rdb[:, :])
            nc.vector.tensor_scalar_mul(tr[:, :], tr[:, :], rden[:, 0:1])
            nc.sync.dma_start(curv_d[:, :], tr[:, :])


def build_nc(n=N):
    nc = bacc.Bacc("TRN2", target_bir_lowering=False, debug=False,
                   enable_asserts=False, num_devices=B)
    ns = n // 128
    nsc = n // 128
    gbt_d = nc.dram_tensor("gbt", [KROWS, n], bf16,
                           kind="ExternalInput").ap()
    gat_d = nc.dram_tensor("gat", [KROWS, n], bf16,
                           kind="ExternalInput").ap()
    gbt2_d = nc.dram_tensor("gbt2", [KROWS2, n], bf16,
                            kind="ExternalInput").ap()
    gat2_d = nc.dram_tensor("gat2", [KROWS2, n], bf16,
                            kind="ExternalInput").ap()
    fp18_d = nc.dram_tensor("fp18", [128, (ns // 2) * 2 * PR], fp8,
                            kind="ExternalInput").ap()
    comb_d = nc.dram_tensor("comb", [PR, 5], f32, kind="ExternalInput").ap()
    id_d = nc.dram_tensor("identb", [128, 128], bf16,
                          kind="ExternalInput").ap()
    c18_d = nc.dram_tensor("c18", [PR, 1], f32, kind="ExternalInput").ap()
    msq_d = nc.dram_tensor("msq", [128, 1], f32, kind="ExternalInput").ap()
    curv_d = nc.dram_tensor("curv", [128, nsc], f32,
                            kind="ExternalOutput").ap()
    cnt_d = nc.dram_tensor("cnt", [128, nsc], f32,
                           kind="ExternalOutput").ap()
    sdbg_d = None
    if DEBUG_S:
        sdbg_d = nc.dram_tensor("sdbg", [PR, n], f32,
                                kind="ExternalOutput").ap()
    with tile.TileContext(nc) as tc:
        build_device_kernel(tc, gbt_d, gat_d, gbt2_d, gat2_d, fp18_d, comb_d,
                            id_d, c18_d, msq_d, curv_d, cnt_d, n=n,
                            sdbg_d=sdbg_d)
    nc.compile()
    return nc


def _split(v):
    """Exact bf16 hi/lo split of a float32/64 vector."""
    h = v.astype(BF16)
    l = (v.astype(np.float64) - h.astype(np.float64)).astype(BF16)
    return h, l


def host_inputs(p, n=N):
    """Per-batch host prep. p: [n, 3] float32 (uncentered)."""
    ns = n // 128
    mu = p.mean(axis=0, dtype=np.float64)
    pc = (p.astype(np.float64) - mu).astype(np.float32)
    xh, xl = _split(pc[:, 0])
    yh, yl = _split(pc[:, 1])
    zh, zl = _split(pc[:, 2])
    x64 = xh.astype(np.float64) + xl.astype(np.float64)
    y64 = yh.astype(np.float64) + yl.astype(np.float64)
    z64 = zh.astype(np.float64) + zl.astype(np.float64)
    sq64 = x64 * x64 + y64 * y64 + z64 * z64
    sh, sl = _split(sq64)

    one = np.ones(n, BF16)
    zero = np.zeros(n, BF16)

    def b2(a):  # exact doubling in bf16
        return (2.0 * a.astype(np.float32)).astype(BF16)

    # pass A (13 rows): moving rows (point j) pair with stationary (point i)
    gat = np.stack([xh, xl, xh, yh, yl, yh, zh, zl, zh,
                    sh, sl, one, one])
    gbt = np.stack([b2(xh), b2(xh), b2(xl), b2(yh), b2(yh), b2(yl),
                    b2(zh), b2(zh), b2(zl),
                    -one, -one, -sh, -sl])
    # pass B (16 rows): role-swapped so the product at (d, q) equals pass
    # A's product at (q, d) bit-exactly, row by row; rows 13..15 add
    # w = -v5[q] via a 3-part bf16 split (filled on device).
    gbt2 = np.stack([b2(xh), b2(xl), b2(xh), b2(yh), b2(yl), b2(yh),
                     b2(zh), b2(zl), b2(zh),
                     -sh, -sl, -one, -one, one, one, one])
    gat2 = np.stack([xh, xh, xl, yh, yh, yl, zh, zh, zl,
                     one, one, sh, sl, zero, zero, zero])

    # masked-sum features in fp8 (DoubleRow): each of [x y z sc] is
    # decomposed into signed BIT-PLANE rows (entries in {0, +-1}), with the
    # plane weight applied by the fp32 `comb` matmul in the finalize.
    # Uniform +-1 magnitudes keep the dual-fp8 PE unit exact (its pair
    # adder truncates the smaller partner when exponents differ), and the
    # +-1 psum sums are exact integers in fp32. sq is centered
    # (sc = sq - mean); the finalize adds cnt * msq back to Ssq.
    msq = float(sq64.mean())

    def planes(v, top, count):
        # signed-digit decomposition, round-to-nearest per plane (unbiased)
        a = v.copy()
        assert np.abs(a).max() < 1.5 * top, (np.abs(a).max(), top)
        rows, weights = [], []
        w = float(top)
        for _ in range(count):
            b = np.clip(np.round(a / w), -1, 1)
            a = a - b * w
            rows.append(b.astype(FP8))
            weights.append(w)
            w *= 0.5
        return rows, weights

    xr, xw = planes(x64, 8.0, PLANES_XYZ)
    yr, yw = planes(y64, 8.0, PLANES_XYZ)
    zr, zw = planes(z64, 8.0, PLANES_XYZ)
    sr, sw = planes(sq64 - msq, 64.0, PLANES_SC)
    rows18 = xr + yr + zr + sr + [np.ones(n, FP8)]
    rows18 += [np.zeros(n, FP8)] * (PR - NROWS)
    f18 = np.stack(rows18, axis=1)                           # [n, PR]
    # d-slab pair layout: pair t cols [0:PR]=slab 2t, [PR:2PR]=slab 2t+1
    f18 = f18.reshape(ns, 128, PR).transpose(1, 0, 2).reshape(128, ns * PR)
    fp18 = np.ascontiguousarray(f18)                         # [128, ns*PR]

    # +-1 correction constants: colsum per digit row (exact integers)
    c18 = np.array([r.astype(np.float64).sum() for r in rows18],
                   dtype=np.float64).astype(np.float32).reshape(PR, 1)
    # combine: S5[c] = sum_r (w_r / 2) * (psum_r + c18_r)
    comb = np.zeros((PR, 5), np.float32)
    wall = [xw, yw, zw, sw]
    ofs = 0
    for c in range(4):
        for i, w in enumerate(wall[c]):
            comb[ofs + i, c] = w * 0.5
        ofs += len(wall[c])
    comb[NROWS - 1, 4] = 0.5
    msqr = np.full((128, 1), msq, dtype=np.float32)

    identb = np.eye(128, dtype=BF16)
    return {"gbt": np.ascontiguousarray(gbt),
            "gat": np.ascontiguousarray(gat),
            "gbt2": np.ascontiguousarray(gbt2),
            "gat2": np.ascontiguousarray(gat2),
            "fp18": fp18, "comb": comb,
            "identb": identb, "c18": c18, "msq": msqr}


_NC_CACHE = {}


def kernel(pcd, k):
    assert int(k) == 5, f"kernel hardcodes k=5, got {k}"
    pcd = np.asarray(pcd, dtype=np.float32)
    assert pcd.shape == (B, N, 3), pcd.shape
    if N not in _NC_CACHE:
        _NC_CACHE[N] = build_nc(N)
    nc = _NC_CACHE[N]
    in_maps = [host_inputs(pcd[b]) for b in range(B)]
    res = run_bass_kernel_spmd(nc, in_maps, core_ids=list(range(B)))
    out = np.stack([r["curv"].reshape(N) for r in res.results])
    return out.astype(np.float32)


if __name__ == "__main__":
    rng = np.random.default_rng(0)
    pcd = rng.standard_normal((B, N, 3)).astype(np.float32)
    out = kernel(pcd, 5)
    print("kernel output", out.shape, out.dtype, out[0, :4])


# revision 64
# speedup vs baseline: 1.0370x; 1.0370x over previous
"""Trainium2 Bass kernel for nn_MC3DAD_ONNX_48146583388946 (retrieval_knn).

Per batch (one NeuronCore per batch, B=8). Distance matmuls run in bf16
with exact hi/lo coordinate splits (bf16 x bf16 products are exact in the
fp32 PSUM accumulation, and matmul cost depends only on moving columns,
not contraction depth):

  - pass A: -d^2 via a 13-row matmul (hi/lo split of the
    ||x||^2 + ||y||^2 - 2 x.y expansion); VectorE max8 -> v5 = 5th-largest
    -d^2 per point.
  - v5 is negated, split into three bf16 parts, transposed on the PE and
    scattered into three extra moving rows; pass B's 16-row matmul with
    ROLE-SWAPPED rows directly produces the TRANSPOSED margin matrix
    margin[d, q] = -d^2(d,q) - v5[q]. The row ordering makes every product
    and partial sum bit-identical to pass A's (the PE accumulates
    sequential fp32), so a tiny delta=1e-6 suffices for inclusive ties.
  - mask = Sign(margin + delta) on ScalarE, written as fp8 +-1; the
    masked sums use fp8 DoubleRow matmuls (two d-slabs per instruction).
    Features are encoded as signed BIT-PLANE rows (entries {0,+-1},
    round-to-nearest digits) so the dual-fp8 PE unit - which truncates the
    smaller pair partner on large exponent gaps - stays exact, and the
    +-1-mask sums are exact integers. Plane weights and the +-1->selection
    conversion live in a small fp32 combine matmul in the finalize.
  - finalize in [128, 32] layout: Ssq = sc-sum + cnt*mean(sq),
    trace = (Ssq - |Sxyz|^2/c)/(c-1), curvature = trace/(sum trace + 1e-8).

Coordinates are centered per batch on the host (translation-invariant
covariance); sq is centered too so every masked-sum colsum is ~0 and the
+-1 trick avoids catastrophic cancellation.

Schedule: pass A of octet k+1 and the w-row prep are software-pipelined
against pass B of octet k (PE: margins + masked sums; ScalarE: masks;
VectorE: max8), with PSUM banks split 3/2/2/1 across the four pools.
"""

import numpy as np
from contextlib import ExitStack

import concourse.bass as bass
import concourse.bacc as bacc
import concourse.mybir as mybir
import concourse.tile as tile
from concourse.bass_utils import run_bass_kernel_spmd

try:
    import ml_dtypes
    BF16 = ml_dtypes.bfloat16
    FP8 = ml_dtypes.float8_e4m3
except ImportError:  # jax ships ml_dtypes
    from jax import numpy as jnp
    BF16 = jnp.bfloat16
    FP8 = jnp.float8_e4m3

f32 = mybir.dt.float32
bf16 = mybir.dt.bfloat16
fp8 = mybir.dt.float8e4
AF = mybir.ActivationFunctionType
ALU = mybir.AluOpType
DR = mybir.MatmulPerfMode.DoubleRow

N = 4096
B = 8
DELTA = 1e-6                   # inclusive-margin nudge (abs, on d^2 scale)
KROWS = 13                     # cdist contraction rows (hi/lo split)
KROWS2 = 16                    # pass-B rows: 13 role-swapped + 3 w rows
POOL_MASKS = False             # PoolE cannot read PSUM on TRN2
PLANES_XYZ = 20                # bit planes 2^3 .. 2^-16 for x/y/z
PLANES_SC = 23                 # bit planes 2^6 .. 2^-16 for centered sq
NROWS = 3 * PLANES_XYZ + PLANES_SC + 1   # 84 digit rows per d-slab
PR = 96                        # padded to x32 (dual-fp8 LDW ISA restriction)
INTERLEAVE = True              # overlap A(oct k+1) with B(oct k)
DEBUG_S = False                # dump raw s_all to an extra output


def build_device_kernel(tc, gbt_d, gat_d, gbt2_d, gat2_d, fp18_d, comb_d,
                        id_d, c18_d, msq_d, curv_d, cnt_d, n=N, sdbg_d=None):
    nc = tc.nc
    ns = n // 128               # 128-point d-slabs (32)
    noct = ns // 8              # octets of slabs (4)
    nsc = n // 128              # finalize cols per partition (32)
    nac = n // 1024             # pass-A chunks per slab (4)
    nqc = n // 512              # 512-wide q-chunks (8)

    with ExitStack() as ctx:
        cpool = ctx.enter_context(tc.tile_pool(name="consts", bufs=1))
        gbt = cpool.tile([KROWS, n], bf16, tag="gbt")
        gat = cpool.tile([KROWS, n], bf16, tag="gat")
        gbt2 = cpool.tile([KROWS2, n], bf16, tag="gbt2")
        gat2 = cpool.tile([KROWS2, n], bf16, tag="gat2")
        fp18 = cpool.tile([128, (ns // 2) * 2 * PR], fp8, tag="fp18")
        comb = cpool.tile([PR, 5], f32, tag="comb")
        identb = cpool.tile([128, 128], bf16, tag="identb")
        c18 = cpool.tile([PR, 1], f32, tag="c18")
        msq = cpool.tile([128, 1], f32, tag="msq")
        s_all = cpool.tile([PR, n], f32, tag="s_all")
        s5 = cpool.tile([5, n], f32, tag="s5")
        dlt = cpool.tile([128, 1], f32, tag="dlt")
        ones128 = cpool.tile([128, 1], f32, tag="ones128")
        ones1 = cpool.tile([1, 128], f32, tag="ones1")
        nc.vector.memset(dlt[:, :], DELTA)
        nc.vector.memset(ones128[:, :], 1.0)
        nc.vector.memset(ones1[:, :], 1.0)

        # pass-A operands first: they gate the very first matmul; the big
        # fp18 load is only needed once pass B starts
        nc.sync.dma_start(gbt[:, :], gbt_d[:, :])
        nc.sync.dma_start(gat[:, :], gat_d[:, :])
        nc.sync.dma_start(gbt2[:, :], gbt2_d[:, :])
        nc.sync.dma_start(gat2[:, :], gat2_d[:, :])
        nc.sync.dma_start(identb[:, :], id_d[:, :])
        nc.sync.dma_start(fp18[:, :], fp18_d[:, :])
        nc.sync.dma_start(comb[:, :], comb_d[:, :])
        nc.sync.dma_start(c18[:, :], c18_d[:, :])
        nc.sync.dma_start(msq[:, :], msq_d[:, :])

        with tc.tile_pool(name="apsum", bufs=3, space="PSUM") as ap, \
             tc.tile_pool(name="mpsum", bufs=2, space="PSUM") as mp, \
             tc.tile_pool(name="spsum", bufs=2, space="PSUM") as sp, \
             tc.tile_pool(name="tpsum", bufs=1, space="PSUM") as tp, \
             tc.tile_pool(name="work", bufs=2) as wp, \
             tc.tile_pool(name="msks", bufs=4) as kp:

            v5oct = {}

            def emit_A_slab(s):
                """-d^2 row slab s -> top-8 -> v5 column in v5oct."""
                oct_, i = s // 8, s % 8
                if i == 0:
                    v5oct[oct_] = wp.tile([128, 64], f32, tag="v5o",
                                          name=f"v5o_{oct_}")
                m8 = wp.tile([128, 8 * 2 * nac], f32, tag="m8",
                             name=f"m8_{s}")
                for c in range(2 * nac):
                    d1 = ap.tile([128, 512], f32, tag="d1",
                                 name=f"d1_{s}_{c}")
                    nc.tensor.matmul(
                        d1[:, :],
                        gbt[0:KROWS, s * 128:(s + 1) * 128],
                        gat[0:KROWS, c * 512:(c + 1) * 512],
                        start=True, stop=True)
                    nc.vector.max(m8[:, c * 8:(c + 1) * 8], d1[:, :])
                nc.vector.max(v5oct[oct_][:, i * 8:(i + 1) * 8], m8[:, :])

            def emit_wprep(oct_):
                """w = -v5 for the octet's 1024 points, split into three
                bf16 parts (exact to 2^-25), transpose, scatter into gat2
                rows 13/14/15."""
                vo = v5oct[oct_]
                wf = wp.tile([128, 8], f32, tag="wf", name=f"wf_{oct_}")
                nc.vector.tensor_scalar_mul(wf[:, :], vo[:, 4:64:8], -1.0)
                wc = wp.tile([128, 24], bf16, tag="wc", name=f"wc_{oct_}")
                whf = wp.tile([128, 8], f32, tag="whf", name=f"whf_{oct_}")
                for part in range(3):
                    nc.vector.tensor_copy(wc[:, part * 8:(part + 1) * 8],
                                          wf[:, :])
                    if part == 2:
                        break
                    nc.vector.tensor_copy(whf[:, :],
                                          wc[:, part * 8:(part + 1) * 8])
                    nc.vector.tensor_sub(wf[:, :], wf[:, :], whf[:, :])
                wT = tp.tile([24, 128], bf16, tag="wT", name=f"wT_{oct_}")
                nc.tensor.transpose(wT[:, :], wc[:, :], identb[:, :])
                wTs = wp.tile([24, 128], bf16, tag="wTs", name=f"wTs_{oct_}")
                nc.vector.tensor_copy(wTs[:, :], wT[:, :])
                # [24, 128] rows (part*8+j) -> [3, 1024] row-major: same
                # linear element order, one DMA
                q0 = oct_ * 1024
                nc.sync.dma_start(gat2[13:16, q0:q0 + 1024], wTs[:, :])

            Scur = {}

            def emit_B_pair(oct_, t):
                """margin chunks for d-slab pair (2t, 2t+1) x both q-chunks
                of the octet, sharing stationaries; two DoubleRow
                masked-sum matmuls."""
                if t == 0:
                    for u in range(2):
                        qc = oct_ * 2 + u
                        Scur[qc] = sp.tile([PR, 512], f32, tag="S",
                                           name=f"S_{qc}")
                msks = {}
                for u in range(2):
                    qc = oct_ * 2 + u
                    msks[u] = kp.tile([128, 1024], fp8, tag="msk",
                                      name=f"msk_{qc}_{t}", bufs=4)
                for g in range(2):
                    d = 2 * t + g
                    for u in range(2):
                        qc = oct_ * 2 + u
                        mg = mp.tile([128, 512], f32, tag="mg",
                                     name=f"mg_{qc}_{d}")
                        nc.tensor.matmul(
                            mg[:, :],
                            gbt2[0:KROWS2, d * 128:(d + 1) * 128],
                            gat2[0:KROWS2, qc * 512:(qc + 1) * 512],
                            start=True, stop=True)
                        nc.scalar.activation(
                            msks[u][:, g * 512:(g + 1) * 512],
                            mg[:, :], AF.Sign, bias=dlt[:, 0:1])
                for u in range(2):
                    qc = oct_ * 2 + u
                    nc.tensor.matmul(
                        Scur[qc][:, :],
                        fp18[:, t * 2 * PR:(t + 1) * 2 * PR].rearrange(
                            "p (two f) -> p two f", two=2),
                        msks[u][:, :].rearrange("p (two f) -> p two f",
                                                two=2),
                        start=(t == 0), stop=(t == ns // 2 - 1),
                        perf_mode=DR)
                if t == ns // 2 - 1:
                    for u in range(2):
                        qc = oct_ * 2 + u
                        nc.scalar.copy(s_all[:, qc * 512:(qc + 1) * 512],
                                       Scur[qc][:, :])

            # ---- software-pipelined schedule ----
            # wprep(k) is emitted inside iteration k-1 right after the last
            # A-slab of octet k, so the w transpose/scatter overlaps the
            # tail of B(oct k-1) instead of stalling the octet boundary.
            for i in range(8):
                emit_A_slab(i)
            emit_wprep(0)
            for oct_ in range(noct):
                npair = ns // 2
                for i in range(8):
                    if INTERLEAVE and oct_ + 1 < noct:
                        emit_A_slab((oct_ + 1) * 8 + i)
                        if i == 7:
                            emit_wprep(oct_ + 1)
                    for t in range(i * npair // 8, (i + 1) * npair // 8):
                        emit_B_pair(oct_, t)
                if not INTERLEAVE and oct_ + 1 < noct:
                    for i in range(8):
                        emit_A_slab((oct_ + 1) * 8 + i)
                    emit_wprep(oct_ + 1)

        # ---------------- finalize ----------------
        if sdbg_d is not None:
            nc.sync.dma_start(sdbg_d[:, :], s_all[:, :])
        with tc.tile_pool(name="fin", bufs=1) as fp, \
             tc.tile_pool(name="fpsum", bufs=2, space="PSUM") as fps:
            # +-1 mask correction: + 0.5 * colsum per digit row, then fold
            # the digit rows into [x y z sq cnt] via a small fp32 matmul.
            # Per-chunk adds let chunks drained early run their correction
            # on VectorE while the PE finishes the last octet.
            for ch in range(nqc):
                sl = slice(ch * 512, (ch + 1) * 512)
                nc.vector.tensor_scalar_add(s_all[0:PR, sl],
                                            s_all[0:PR, sl], c18[0:PR, 0:1])
            # reshape rows into [128, nsc] (q = p * nsc + c, row-major)
            fin = fp.tile([128, 5 * nsc], f32, tag="fin")
            for ch in range(nqc):
                fc = fps.tile([5, 512], f32, tag="fc", name=f"fc_{ch}")
                nc.tensor.matmul(fc[:, :], comb[:, :],
                                 s_all[:, ch * 512:(ch + 1) * 512],
                                 start=True, stop=True)
                nc.scalar.copy(s5[:, ch * 512:(ch + 1) * 512], fc[:, :])
            for r in range(5):
                nc.sync.dma_start(fin[:, r * nsc:(r + 1) * nsc],
                                  s5[r:r + 1, :])
            xb = fin[:, 0 * nsc:1 * nsc]
            yb = fin[:, 1 * nsc:2 * nsc]
            zb = fin[:, 2 * nsc:3 * nsc]
            sqb = fin[:, 3 * nsc:4 * nsc]
            cntb = fin[:, 4 * nsc:5 * nsc]

            # Ssq = sc-sum + cnt * msq (sq was centered host-side)
            tmp = fp.tile([128, nsc], f32, tag="tmp")
            nc.vector.tensor_scalar_mul(tmp[:, :], cntb, msq[:, 0:1])
            nc.vector.tensor_add(sqb, sqb, tmp[:, :])

            qq = fp.tile([128, nsc], f32, tag="qq")
            nc.vector.tensor_mul(qq[:, :], xb, xb)
            nc.vector.tensor_mul(tmp[:, :], yb, yb)
            nc.vector.tensor_add(qq[:, :], qq[:, :], tmp[:, :])
            nc.vector.tensor_mul(tmp[:, :], zb, zb)
            nc.vector.tensor_add(qq[:, :], qq[:, :], tmp[:, :])

            nc.sync.dma_start(cnt_d[:, :], cntb)
            rc = fp.tile([128, nsc], f32, tag="rc")
            rc1 = fp.tile([128, nsc], f32, tag="rc1")
            nc.vector.reciprocal(rc[:, :], cntb)
            cm1 = fp.tile([128, nsc], f32, tag="cm1")
            nc.vector.tensor_scalar_add(cm1[:, :], cntb, -1.0)
            nc.vector.reciprocal(rc1[:, :], cm1[:, :])

            tr = fp.tile([128, nsc], f32, tag="tr")
            nc.vector.tensor_mul(qq[:, :], qq[:, :], rc[:, :])
            nc.vector.tensor_sub(tr[:, :], sqb, qq[:, :])
            nc.vector.tensor_mul(tr[:, :], tr[:, :], rc1[:, :])

            red = fp.tile([128, 1], f32, tag="red")
            nc.vector.reduce_sum(red[:, :], tr[:, :],
                                 axis=mybir.AxisListType.X)
            # partition sum + broadcast via two tiny matmuls (avoids the
            # ~7us GpSimd library reload of partition_all_reduce)
            tot = fps.tile([1, 1], f32, tag="tot")
            nc.tensor.matmul(tot[:, :], ones128[:, :], red[:, :],
                             start=True, stop=True)
            tots = fp.tile([1, 1], f32, tag="tots")
            nc.vector.tensor_scalar_add(tots[:, :], tot[:, :], 1e-8)
            rde = fp.tile([1, 1], f32, tag="rde")
            nc.vector.reciprocal(rde[:, :], tots[:, :])
            rdb = fps.tile([128, 1], f32, tag="rdb")
            nc.tensor.matmul(rdb[:, :], ones1[:, :], rde[:, :],
                             start=True, stop=True)
            rden = fp.tile([128, 1], f32, tag="rden")
            nc.vector.tensor_copy(rden[:, :], rdb[:, :])
            nc.vector.tensor_scalar_mul(tr[:, :], tr[:, :], rden[:, 0:1])
            nc.sync.dma_start(curv_d[:, :], tr[:, :])


def build_nc(n=N):
    nc = bacc.Bacc("TRN2", target_bir_lowering=False, debug=False,
                   enable_asserts=False, num_devices=B)
    ns = n // 128
    nsc = n // 128
    gbt_d = nc.dram_tensor("gbt", [KROWS, n], bf16,
                           kind="ExternalInput").ap()
    gat_d = nc.dram_tensor("gat", [KROWS, n], bf16,
                           kind="ExternalInput").ap()
    gbt2_d = nc.dram_tensor("gbt2", [KROWS2, n], bf16,
                            kind="ExternalInput").ap()
    gat2_d = nc.dram_tensor("gat2", [KROWS2, n], bf16,
                            kind="ExternalInput").ap()
    fp18_d = nc.dram_tensor("fp18", [128, (ns // 2) * 2 * PR], fp8,
                            kind="ExternalInput").ap()
    comb_d = nc.dram_tensor("comb", [PR, 5], f32, kind="ExternalInput").ap()
    id_d = nc.dram_tensor("identb", [128, 128], bf16,
                          kind="ExternalInput").ap()
    c18_d = nc.dram_tensor("c18", [PR, 1], f32, kind="ExternalInput").ap()
    msq_d = nc.dram_tensor("msq", [128, 1], f32, kind="ExternalInput").ap()
    curv_d = nc.dram_tensor("curv", [128, nsc], f32,
                            kind="ExternalOutput").ap()
    cnt_d = nc.dram_tensor("cnt", [128, nsc], f32,
                           kind="ExternalOutput").ap()
    sdbg_d = None
    if DEBUG_S:
        sdbg_d = nc.dram_tensor("sdbg", [PR, n], f32,
                                kind="ExternalOutput").ap()
    with tile.TileContext(nc) as tc:
        build_device_kernel(tc, gbt_d, gat_d, gbt2_d, gat2_d, fp18_d, comb_d,
                            id_d, c18_d, msq_d, curv_d, cnt_d, n=n,
                            sdbg_d=sdbg_d)
    nc.compile()
    return nc


def _split(v):
    """Exact bf16 hi/lo split of a float32/64 vector."""
    h = v.astype(BF16)
    l = (v.astype(np.float64) - h.astype(np.float64)).astype(BF16)
    return h, l


def host_inputs(p, n=N):
    """Per-batch host prep. p: [n, 3] float32 (uncentered)."""
    ns = n // 128
    mu = p.mean(axis=0, dtype=np.float64)
    pc = (p.astype(np.float64) - mu).astype(np.float32)
    xh, xl = _split(pc[:, 0])
    yh, yl = _split(pc[:, 1])
    zh, zl = _split(pc[:, 2])
    x64 = xh.astype(np.float64) + xl.astype(np.float64)
    y64 = yh.astype(np.float64) + yl.astype(np.float64)
    z64 = zh.astype(np.float64) + zl.astype(np.float64)
    sq64 = x64 * x64 + y64 * y64 + z64 * z64
    sh, sl = _split(sq64)

    one = np.ones(n, BF16)
    zero = np.zeros(n, BF16)

    def b2(a):  # exact doubling in bf16
        return (2.0 * a.astype(np.float32)).astype(BF16)

    # pass A (13 rows): moving rows (point j) pair with stationary (point i)
    gat = np.stack([xh, xl, xh, yh, yl, yh, zh, zl, zh,
                    sh, sl, one, one])
    gbt = np.stack([b2(xh), b2(xh), b2(xl), b2(yh), b2(yh), b2(yl),
                    b2(zh), b2(zh), b2(zl),
                    -one, -one, -sh, -sl])
    # pass B (16 rows): role-swapped so the product at (d, q) equals pass
    # A's product at (q, d) bit-exactly, row by row; rows 13..15 add
    # w = -v5[q] via a 3-part bf16 split (filled on device).
    gbt2 = np.stack([b2(xh), b2(xl), b2(xh), b2(yh), b2(yl), b2(yh),
                     b2(zh), b2(zl), b2(zh),
                     -sh, -sl, -one, -one, one, one, one])
    gat2 = np.stack([xh, xh, xl, yh, yh, yl, zh, zh, zl,
                     one, one, sh, sl, zero, zero, zero])

    # masked-sum features in fp8 (DoubleRow): each of [x y z sc] is
    # decomposed into signed BIT-PLANE rows (entries in {0, +-1}), with the
    # plane weight applied by the fp32 `comb` matmul in the finalize.
    # Uniform +-1 magnitudes keep the dual-fp8 PE unit exact (its pair
    # adder truncates the smaller partner when exponents differ), and the
    # +-1 psum sums are exact integers in fp32. sq is centered
    # (sc = sq - mean); the finalize adds cnt * msq back to Ssq.
    msq = float(sq64.mean())

    def planes(v, top, count):
        # signed-digit decomposition, round-to-nearest per plane (unbiased)
        a = v.copy()
        assert np.abs(a).max() < 1.5 * top, (np.abs(a).max(), top)
        rows, weights = [], []
        w = float(top)
        for _ in range(count):
            b = np.clip(np.round(a / w), -1, 1)
            a = a - b * w
            rows.append(b.astype(FP8))
            weights.append(w)
            w *= 0.5
        return rows, weights

    xr, xw = planes(x64, 8.0, PLANES_XYZ)
    yr, yw = planes(y64, 8.0, PLANES_XYZ)
    zr, zw = planes(z64, 8.0, PLANES_XYZ)
    sr, sw = planes(sq64 - msq, 64.0, PLANES_SC)
    rows18 = xr + yr + zr + sr + [np.ones(n, FP8)]
    rows18 += [np.zeros(n, FP8)] * (PR - NROWS)
    f18 = np.stack(rows18, axis=1)                           # [n, PR]
    # d-slab pair layout: pair t cols [0:PR]=slab 2t, [PR:2PR]=slab 2t+1
    f18 = f18.reshape(ns, 128, PR).transpose(1, 0, 2).reshape(128, ns * PR)
    fp18 = np.ascontiguousarray(f18)                         # [128, ns*PR]

    # +-1 correction constants: colsum per digit row (exact integers)
    c18 = np.array([r.astype(np.float64).sum() for r in rows18],
                   dtype=np.float64).astype(np.float32).reshape(PR, 1)
    # combine: S5[c] = sum_r (w_r / 2) * (psum_r + c18_r)
    comb = np.zeros((PR, 5), np.float32)
    wall = [xw, yw, zw, sw]
    ofs = 0
    for c in range(4):
        for i, w in enumerate(wall[c]):
            comb[ofs + i, c] = w * 0.5
        ofs += len(wall[c])
    comb[NROWS - 1, 4] = 0.5
    msqr = np.full((128, 1), msq, dtype=np.float32)

    identb = np.eye(128, dtype=BF16)
    return {"gbt": np.ascontiguousarray(gbt),
            "gat": np.ascontiguousarray(gat),
            "gbt2": np.ascontiguousarray(gbt2),
            "gat2": np.ascontiguousarray(gat2),
            "fp18": fp18, "comb": comb,
            "identb": identb, "c18": c18, "msq": msqr}


_NC_CACHE = {}


def kernel(pcd, k):
    assert int(k) == 5, f"kernel hardcodes k=5, got {k}"
    pcd = np.asarray(pcd, dtype=np.float32)
    assert pcd.shape == (B, N, 3), pcd.shape
    if N not in _NC_CACHE:
        _NC_CACHE[N] = build_nc(N)
    nc = _NC_CACHE[N]
    in_maps = [host_inputs(pcd[b]) for b in range(B)]
    res = run_bass_kernel_spmd(nc, in_maps, core_ids=list(range(B)))
    out = np.stack([r["curv"].reshape(N) for r in res.results])
    return out.astype(np.float32)


if __name__ == "__main__":
    rng = np.random.default_rng(0)
    pcd = rng.standard_normal((B, N, 3)).astype(np.float32)
    out = kernel(pcd, 5)
    print("kernel output", out.shape, out.dtype, out[0, :4])
